# revision 14
# baseline (speedup 1.0000x reference)
"""Trainium2 Bass kernel for MoE head adapter (top-2 of 4 experts + proj).

Computes, for full inputs x[65536,256], w_gate[256,4], w1[4,256,512],
w2[4,512,256], w_proj[256,512], b_proj[512]:

    logits = x @ w_gate; top-2 softmax gates
    h = gelu(x @ w1[e]); y = sum_e g_e * (h_e @ w2[e]); out = y @ w_proj + b_proj

Sharding: pure data-parallel over tokens across 8 NeuronCores (8192
tokens/core, weights replicated, no collectives).

Per-core structure (two phases, to keep the PE HAM clock warm and avoid
ACT activation-table reloads):
  Phase A: for all super-tiles: load x, PE-transpose to xT (f32r + f32
           copies), gating logits (exact f32), top-2 softmax gates,
           per-expert gate rows gT[e].
  Phase B: for all super-tiles: dense 4-expert up-proj / gelu*gate /
           down-proj accumulation (transposed orientation, 512-token
           moving operands, float32r) + output projection.
"""

import os
from contextlib import ExitStack

import numpy as np

import concourse.bass as bass
import concourse.tile as tile
from concourse import bacc, mybir
from concourse.bass_utils import run_bass_kernel_spmd

N, D, E, H, EMB = 65536, 256, 4, 512, 512
NCORES = 8
NSH = N // NCORES          # tokens per core
SUPER = 512                # tokens per super-tile
NSUP = NSH // SUPER
S_BLK = SUPER // 128       # 128-token sub-blocks per super-tile
KD = D // 128              # k-tiles over D
MH = H // 128              # m-tiles over H

F32 = mybir.dt.float32
AF = mybir.ActivationFunctionType
ALU = mybir.AluOpType
AX = mybir.AxisListType

# bf16 matmul operands: 1 cy/row on the PE + automatic fast-weight-load.
MM_DT = mybir.dt.bfloat16


def _moe_body(ctx: ExitStack, tc, xt, wg, w1, w2, wp, ident, out):
    nc = tc.nc

    const = ctx.enter_context(tc.tile_pool(name="const", bufs=1))
    keep = ctx.enter_context(tc.tile_pool(name="keep", bufs=1))
    sb = ctx.enter_context(tc.tile_pool(name="sb", bufs=2))
    ps_big = ctx.enter_context(tc.tile_pool(name="psbig", bufs=5, space="PSUM"))
    ps_yt = ctx.enter_context(tc.tile_pool(name="psyt", bufs=2, space="PSUM"))
    ps_sm = ctx.enter_context(tc.tile_pool(name="pssm", bufs=1, space="PSUM"))

    # --- replicated constants -------------------------------------------------
    w1_sb = const.tile([128, KD, E, H], MM_DT)
    w2_sb = const.tile([128, MH, E, D], MM_DT)
    wp_sb = const.tile([128, KD, EMB], MM_DT)
    wg_sb = const.tile([128, KD, E], F32)
    for k in range(KD):
        for e in range(E):
            nc.sync.dma_start(
                w1_sb[:, k, e, :], w1[e, k * 128 : (k + 1) * 128, :]
            )
    for m in range(MH):
        for e in range(E):
            nc.sync.dma_start(
                w2_sb[:, m, e, :], w2[e, m * 128 : (m + 1) * 128, :]
            )
    nc.sync.dma_start(
        wp_sb[:], wp.rearrange("(k p) m -> p k m", p=128)
    )
    nc.sync.dma_start(wg_sb[:], wg.rearrange("(k p) e -> p k e", p=128))
    id_sb = const.tile([128, 128], F32)
    nc.sync.dma_start(id_sb[:], ident[:])
    ones_sb = const.tile([1, 128], MM_DT)
    nc.vector.memset(ones_sb[:], 1.0)

    # persistent across phases
    xt_all = keep.tile([128, NSUP, KD, SUPER], MM_DT)
    gte_dram = nc.dram_tensor("gte_scratch", [NSUP, E, SUPER], MM_DT).ap()

    # ======================= Phase A: gating =================================
    for T in range(NSUP):
        tok0 = T * SUPER

        xt32_sb = sb.tile([128, KD, SUPER], F32, tag="xt32", bufs=4)
        nc.sync.dma_start(
            xt32_sb[:],
            xt[:, tok0 : tok0 + SUPER].rearrange("(k p) t -> p k t", p=128),
        )
        nc.scalar.copy(xt_all[:, T, :, :], xt32_sb[:])

        # gating logits [tok, s, e] in exact f32
        lg_ps = ps_sm.tile([128, S_BLK, E], F32, tag="sm")
        for s in range(S_BLK):
            for k in range(KD):
                nc.tensor.matmul(
                    lg_ps[:, s, :],
                    xt32_sb[:, k, s * 128 : (s + 1) * 128],
                    wg_sb[:, k, :],
                    start=(k == 0),
                    stop=(k == KD - 1),
                )

        # top-2 softmax gates
        def bc(t):
            return t[:].broadcast_to([128, S_BLK, E])

        lg = sb.tile([128, S_BLK, E], F32, tag="lg")
        nc.vector.tensor_copy(lg[:], lg_ps[:])
        m1 = sb.tile([128, S_BLK, 1], F32, tag="m1")
        nc.vector.reduce_max(m1[:], lg[:], axis=AX.X)
        t0 = sb.tile([128, S_BLK, E], F32, tag="t0")
        nc.vector.tensor_tensor(t0[:], lg[:], bc(m1), op=ALU.is_equal)
        t1 = sb.tile([128, S_BLK, E], F32, tag="t1")
        nc.vector.tensor_scalar_mul(t1[:], t0[:], -1e9)
        t2 = sb.tile([128, S_BLK, E], F32, tag="t2")
        nc.vector.tensor_tensor(t2[:], lg[:], t1[:], op=ALU.add)
        m2 = sb.tile([128, S_BLK, 1], F32, tag="m2")
        nc.vector.reduce_max(m2[:], t2[:], axis=AX.X)
        t3 = sb.tile([128, S_BLK, E], F32, tag="t3")
        nc.vector.tensor_tensor(t3[:], lg[:], bc(m2), op=ALU.is_ge)
        t4 = sb.tile([128, S_BLK, E], F32, tag="t4")
        nc.vector.tensor_tensor(t4[:], lg[:], bc(m1), op=ALU.subtract)
        t5 = sb.tile([128, S_BLK, E], F32, tag="t5")
        nc.scalar.activation(t5[:], t4[:], AF.Exp)
        t6 = sb.tile([128, S_BLK, E], F32, tag="t6")
        nc.vector.tensor_tensor(t6[:], t5[:], t3[:], op=ALU.mult)
        den = sb.tile([128, S_BLK, 1], F32, tag="den")
        nc.vector.reduce_sum(den[:], t6[:], axis=AX.X)
        rcp = sb.tile([128, S_BLK, 1], F32, tag="rcp")
        nc.vector.reciprocal(rcp[:], den[:])
        g_sb = sb.tile([128, S_BLK, E], F32, tag="g")
        nc.vector.tensor_tensor(g_sb[:], t6[:], bc(rcp), op=ALU.mult)

        # gT [e, tok] via one PE transpose per s-block, then partition-shift
        # DMAs to get each expert row based at partition 0.
        gt_ps = ps_sm.tile([E, SUPER], F32, tag="sm")
        for s in range(S_BLK):
            nc.tensor.transpose(
                gt_ps[:, s * 128 : (s + 1) * 128], g_sb[:, s, :], id_sb[:]
            )
        gt_sb = sb.tile([E, SUPER], MM_DT, tag="gtsb")
        nc.vector.tensor_copy(gt_sb[:], gt_ps[:])
        for e in range(E):
            nc.sync.dma_start(gte_dram[T, e, :], gt_sb[e : e + 1, :])

    # ======================= Phase B: experts + proj =========================
    for T in range(NSUP):
        tok0 = T * SUPER

        yt_ps = [
            ps_yt.tile([128, SUPER], F32, tag="yt", name=f"yt{T}_{md}")
            for md in range(KD)
        ]
        hgg_all = []

        def _down(e):
            hgg_e = hgg_all[e]
            for md in range(KD):
                for m in range(MH):
                    nc.tensor.matmul(
                        yt_ps[md][:],
                        w2_sb[:, m, e, md * 128 : (md + 1) * 128],
                        hgg_e[:, m, :],
                        start=(e == 0 and m == 0),
                        stop=(e == E - 1 and m == MH - 1),
                    )

        for e in range(E):
            gte_sb = sb.tile([1, SUPER], MM_DT, tag="gte")
            nc.sync.dma_start(gte_sb[:], gte_dram[T, e, :][None, :])
            G_ps = ps_big.tile([128, SUPER], F32, tag="big", name=f"G{T}_{e}")
            nc.tensor.matmul(
                G_ps[:], ones_sb[:], gte_sb[:], start=True, stop=True
            )
            hgg = sb.tile(
                [128, MH, SUPER], MM_DT, tag="hgg", name=f"hgg{T}_{e}", bufs=3
            )
            hgg_all.append(hgg)
            for m in range(MH):
                h_ps = ps_big.tile([128, SUPER], F32, tag="big")
                for k in range(KD):
                    nc.tensor.matmul(
                        h_ps[:],
                        w1_sb[:, k, e, m * 128 : (m + 1) * 128],
                        xt_all[:, T, k, :],
                        start=(k == 0),
                        stop=(k == KD - 1),
                    )
                hg = sb.tile([128, SUPER], F32, tag="hg")
                nc.scalar.activation(hg[:], h_ps[:], AF.Gelu)
                nc.vector.tensor_mul(hgg[:, m, :], hg[:], G_ps[:])
            if e > 1:
                _down(e - 2)
        _down(E - 2)
        _down(E - 1)
        yt_sb = sb.tile([128, KD, SUPER], MM_DT, tag="ytsb")
        for md in range(KD):
            nc.vector.tensor_copy(yt_sb[:, md, :], yt_ps[md][:])

        for s in range(S_BLK):
            o_ps = ps_big.tile([128, EMB], F32, tag="big")
            for kd in range(KD):
                nc.tensor.matmul(
                    o_ps[:],
                    yt_sb[:, kd, s * 128 : (s + 1) * 128],
                    wp_sb[:, kd, :],
                    start=(kd == 0),
                    stop=(kd == KD - 1),
                )
            o_sb = sb.tile([128, EMB], F32, tag="osb")
            if s % 4 != 3:
                nc.scalar.copy(o_sb[:], o_ps[:])
            else:
                nc.vector.tensor_copy(o_sb[:], o_ps[:])
            nc.sync.dma_start(out[tok0 + s * 128 : tok0 + (s + 1) * 128, :], o_sb[:])


_PROGRAM = None


def _build():
    global _PROGRAM
    if _PROGRAM is not None:
        return _PROGRAM
    nc = bacc.Bacc("TRN2", target_bir_lowering=False, debug=False, num_devices=NCORES)
    xt = nc.dram_tensor("xt", [D, NSH], F32, kind="ExternalInput").ap()
    wg = nc.dram_tensor("w_gate", [D, E], F32, kind="ExternalInput").ap()
    w1 = nc.dram_tensor("w1", [E, D, H], MM_DT, kind="ExternalInput").ap()
    w2 = nc.dram_tensor("w2", [E, H, D], MM_DT, kind="ExternalInput").ap()
    wp = nc.dram_tensor("w_proj", [D, EMB], MM_DT, kind="ExternalInput").ap()
    ident = nc.dram_tensor("ident", [128, 128], F32, kind="ExternalInput").ap()
    out = nc.dram_tensor("out", [NSH, EMB], F32, kind="ExternalOutput").ap()
    with tile.TileContext(nc) as tc, ExitStack() as ctx:
        _moe_body(ctx, tc, xt, wg, w1, w2, wp, ident, out)
    nc.compile()
    _PROGRAM = nc
    return nc


def _install_trace_shim():
    """Recreate the antenv.axon_hooks NTFF profile hook (missing in this image)."""
    import sys
    import types
    import contextlib
    import ctypes

    if "antenv.axon_hooks" in sys.modules:
        return
    so_path = "/opt/axon/libaxon_pjrt.so"
    lib = ctypes.CDLL(so_path)
    lib.axon_start_nrt_profile.argtypes = [ctypes.POINTER(ctypes.c_int64), ctypes.c_size_t]
    lib.axon_start_nrt_profile.restype = ctypes.c_int64
    lib.axon_stop_nrt_profile.argtypes = [ctypes.c_char_p]
    lib.axon_stop_nrt_profile.restype = ctypes.c_int64

    @contextlib.contextmanager
    def _hook(output_dir, device_ids):
        import jax

        jax.devices()
        if device_ids:
            ids = (ctypes.c_int64 * len(device_ids))(*device_ids)
            rc = lib.axon_start_nrt_profile(ids, len(device_ids))
        else:
            rc = lib.axon_start_nrt_profile(None, 0)
        if rc != 0:
            raise RuntimeError(f"axon_start_nrt_profile rc={rc}")
        try:
            yield
        finally:
            n = lib.axon_stop_nrt_profile(str(output_dir).encode())
            if n <= 0:
                print(f"profile: {n} ntff files written to {output_dir}")

    mod = types.ModuleType("antenv.axon_hooks")
    _state = {"hook": _hook}
    mod.get_axon_ntff_profile_hook = lambda: _state["hook"]
    mod.set_axon_ntff_profile_hook = lambda h: _state.__setitem__("hook", h)
    sys.modules["antenv.axon_hooks"] = mod

    import concourse.bass_utils as bu

    bu.upload_artifacts = lambda tmpdir: f"local:{tmpdir}"


def kernel(x, w_gate, w1, w2, w_proj, b_proj):
    nc = _build()
    import ml_dtypes

    bf16 = ml_dtypes.bfloat16
    ident = np.eye(128, dtype=np.float32)
    w1_b = np.ascontiguousarray(w1.astype(bf16))
    w2_b = np.ascontiguousarray(w2.astype(bf16))
    wp_b = np.ascontiguousarray(w_proj.astype(bf16))
    in_maps = [
        {
            "xt": np.ascontiguousarray(x[i * NSH : (i + 1) * NSH].T),
            "w_gate": np.ascontiguousarray(w_gate),
            "w1": w1_b,
            "w2": w2_b,
            "w_proj": wp_b,
            "ident": ident,
        }
        for i in range(NCORES)
    ]
    trace = bool(int(os.environ.get("MOE_TRACE", "0")))
    if trace:
        _install_trace_shim()
        import tempfile

        tmpdir = os.environ.get("MOE_TRACE_DIR") or tempfile.mkdtemp(prefix="moe_trace_")
        res = run_bass_kernel_spmd(
            nc, in_maps, list(range(NCORES)), trace=True, tmpdir=tmpdir,
            trace_cores=[0],
        )
        print(f"HW exec time: {res.exec_time_ns} ns")
        print(f"trace dir: {tmpdir}")
        kernel.last_result = res
    else:
        res = run_bass_kernel_spmd(nc, in_maps, list(range(NCORES)))
    full = np.concatenate([res.results[i]["out"] for i in range(NCORES)], axis=0)
    return full + b_proj[None, :]


# revision 15
# speedup vs baseline: 1.0649x; 1.0649x over previous
"""Trainium2 Bass kernel for MoE head adapter (top-2 of 4 experts + proj).

Computes, for full inputs x[65536,256], w_gate[256,4], w1[4,256,512],
w2[4,512,256], w_proj[256,512], b_proj[512]:

    logits = x @ w_gate; top-2 softmax gates
    h = gelu(x @ w1[e]); y = sum_e g_e * (h_e @ w2[e]); out = y @ w_proj + b_proj

Sharding: pure data-parallel over tokens across 8 NeuronCores (8192
tokens/core, weights replicated, no collectives).

Per-core structure (two phases, to keep the PE HAM clock warm and avoid
ACT activation-table reloads):
  Phase A: for all super-tiles: load x, PE-transpose to xT (f32r + f32
           copies), gating logits (exact f32), top-2 softmax gates,
           per-expert gate rows gT[e].
  Phase B: for all super-tiles: dense 4-expert up-proj / gelu*gate /
           down-proj accumulation (transposed orientation, 512-token
           moving operands, float32r) + output projection.
"""

import os
from contextlib import ExitStack

import numpy as np

import concourse.bass as bass
import concourse.tile as tile
from concourse import bacc, mybir
from concourse.bass_utils import run_bass_kernel_spmd

N, D, E, H, EMB = 65536, 256, 4, 512, 512
NCORES = 8
NSH = N // NCORES          # tokens per core
SUPER = 512                # tokens per super-tile
NSUP = NSH // SUPER
S_BLK = SUPER // 128       # 128-token sub-blocks per super-tile
KD = D // 128              # k-tiles over D
MH = H // 128              # m-tiles over H

F32 = mybir.dt.float32
AF = mybir.ActivationFunctionType
ALU = mybir.AluOpType
AX = mybir.AxisListType

# bf16 matmul operands: 1 cy/row on the PE + automatic fast-weight-load.
MM_DT = mybir.dt.bfloat16


def _moe_body(ctx: ExitStack, tc, xt, wg, w1, w2, wp, ident, out):
    nc = tc.nc

    const = ctx.enter_context(tc.tile_pool(name="const", bufs=1))
    keep = ctx.enter_context(tc.tile_pool(name="keep", bufs=1))
    sb = ctx.enter_context(tc.tile_pool(name="sb", bufs=2))
    ps_big = ctx.enter_context(tc.tile_pool(name="psbig", bufs=5, space="PSUM"))
    ps_yt = ctx.enter_context(tc.tile_pool(name="psyt", bufs=2, space="PSUM"))
    ps_sm = ctx.enter_context(tc.tile_pool(name="pssm", bufs=1, space="PSUM"))

    # --- replicated constants -------------------------------------------------
    w1_sb = const.tile([128, KD, E, H], MM_DT)
    w2_sb = const.tile([128, MH, E, D], MM_DT)
    wp_sb = const.tile([128, KD, EMB], MM_DT)
    wg_sb = const.tile([128, KD, E], F32)
    for k in range(KD):
        for e in range(E):
            nc.sync.dma_start(
                w1_sb[:, k, e, :], w1[e, k * 128 : (k + 1) * 128, :]
            )
    for m in range(MH):
        for e in range(E):
            nc.sync.dma_start(
                w2_sb[:, m, e, :], w2[e, m * 128 : (m + 1) * 128, :]
            )
    nc.sync.dma_start(
        wp_sb[:], wp.rearrange("(k p) m -> p k m", p=128)
    )
    nc.sync.dma_start(wg_sb[:], wg.rearrange("(k p) e -> p k e", p=128))
    id_sb = const.tile([128, 128], F32)
    nc.sync.dma_start(id_sb[:], ident[:])
    ones_sb = const.tile([1, 128], MM_DT)
    nc.vector.memset(ones_sb[:], 1.0)

    # persistent across phases
    xt_all = keep.tile([128, NSUP, KD, SUPER], MM_DT)
    gte_dram = nc.dram_tensor("gte_scratch", [NSUP, E, SUPER], MM_DT).ap()

    # ======================= Phase A: gating =================================
    for T in range(NSUP):
        tok0 = T * SUPER

        xt32_sb = sb.tile([128, KD, SUPER], F32, tag="xt32", bufs=4)
        nc.sync.dma_start(
            xt32_sb[:],
            xt[:, tok0 : tok0 + SUPER].rearrange("(k p) t -> p k t", p=128),
        )
        nc.vector.tensor_copy(xt_all[:, T, :, :], xt32_sb[:])

        # gating logits [tok, s, e] in exact f32
        lg_ps = ps_sm.tile([128, S_BLK, E], F32, tag="sm")
        for s in range(S_BLK):
            for k in range(KD):
                nc.tensor.matmul(
                    lg_ps[:, s, :],
                    xt32_sb[:, k, s * 128 : (s + 1) * 128],
                    wg_sb[:, k, :],
                    start=(k == 0),
                    stop=(k == KD - 1),
                )

        # top-2 softmax gates
        def bc(t):
            return t[:].broadcast_to([128, S_BLK, E])

        lg = sb.tile([128, S_BLK, E], F32, tag="lg")
        nc.vector.tensor_copy(lg[:], lg_ps[:])
        m1 = sb.tile([128, S_BLK, 1], F32, tag="m1")
        nc.vector.reduce_max(m1[:], lg[:], axis=AX.X)
        t0 = sb.tile([128, S_BLK, E], F32, tag="t0")
        nc.vector.tensor_tensor(t0[:], lg[:], bc(m1), op=ALU.is_equal)
        t1 = sb.tile([128, S_BLK, E], F32, tag="t1")
        nc.vector.tensor_scalar_mul(t1[:], t0[:], -1e9)
        t2 = sb.tile([128, S_BLK, E], F32, tag="t2")
        nc.vector.tensor_tensor(t2[:], lg[:], t1[:], op=ALU.add)
        m2 = sb.tile([128, S_BLK, 1], F32, tag="m2")
        nc.vector.reduce_max(m2[:], t2[:], axis=AX.X)
        t3 = sb.tile([128, S_BLK, E], F32, tag="t3")
        nc.vector.tensor_tensor(t3[:], lg[:], bc(m2), op=ALU.is_ge)
        t4 = sb.tile([128, S_BLK, E], F32, tag="t4")
        nc.vector.tensor_tensor(t4[:], lg[:], bc(m1), op=ALU.subtract)
        t5 = sb.tile([128, S_BLK, E], F32, tag="t5")
        nc.scalar.activation(t5[:], t4[:], AF.Exp)
        t6 = sb.tile([128, S_BLK, E], F32, tag="t6")
        nc.vector.tensor_tensor(t6[:], t5[:], t3[:], op=ALU.mult)
        den = sb.tile([128, S_BLK, 1], F32, tag="den")
        nc.vector.reduce_sum(den[:], t6[:], axis=AX.X)
        rcp = sb.tile([128, S_BLK, 1], F32, tag="rcp")
        nc.vector.reciprocal(rcp[:], den[:])
        g_sb = sb.tile([128, S_BLK, E], F32, tag="g")
        nc.vector.tensor_tensor(g_sb[:], t6[:], bc(rcp), op=ALU.mult)

        # gT [e, tok] via one PE transpose per s-block, then partition-shift
        # DMAs to get each expert row based at partition 0.
        gt_ps = ps_sm.tile([E, SUPER], F32, tag="sm")
        for s in range(S_BLK):
            nc.tensor.transpose(
                gt_ps[:, s * 128 : (s + 1) * 128], g_sb[:, s, :], id_sb[:]
            )
        gt_sb = sb.tile([E, SUPER], MM_DT, tag="gtsb")
        nc.vector.tensor_copy(gt_sb[:], gt_ps[:])
        for e in range(E):
            nc.sync.dma_start(gte_dram[T, e, :], gt_sb[e : e + 1, :])

    # ======================= Phase B: experts + proj =========================
    for T in range(NSUP):
        tok0 = T * SUPER

        yt_ps = [
            ps_yt.tile([128, SUPER], F32, tag="yt", name=f"yt{T}_{md}")
            for md in range(KD)
        ]
        hgg_all = []

        def _down(e):
            hgg_e = hgg_all[e]
            for md in range(KD):
                for m in range(MH):
                    nc.tensor.matmul(
                        yt_ps[md][:],
                        w2_sb[:, m, e, md * 128 : (md + 1) * 128],
                        hgg_e[:, m, :],
                        start=(e == 0 and m == 0),
                        stop=(e == E - 1 and m == MH - 1),
                    )

        for e in range(E):
            gte_sb = sb.tile([1, SUPER], MM_DT, tag="gte")
            nc.sync.dma_start(gte_sb[:], gte_dram[T, e, :][None, :])
            G_ps = ps_big.tile([128, SUPER], F32, tag="big", name=f"G{T}_{e}")
            nc.tensor.matmul(
                G_ps[:], ones_sb[:], gte_sb[:], start=True, stop=True
            )
            hgg = sb.tile([128, MH, SUPER], MM_DT, tag="hgg", name=f"hgg{T}_{e}")
            hgg_all.append(hgg)
            for m in range(MH):
                h_ps = ps_big.tile([128, SUPER], F32, tag="big")
                for k in range(KD):
                    nc.tensor.matmul(
                        h_ps[:],
                        w1_sb[:, k, e, m * 128 : (m + 1) * 128],
                        xt_all[:, T, k, :],
                        start=(k == 0),
                        stop=(k == KD - 1),
                    )
                hg = sb.tile([128, SUPER], F32, tag="hg")
                nc.scalar.activation(hg[:], h_ps[:], AF.Gelu)
                nc.vector.tensor_mul(hgg[:, m, :], hg[:], G_ps[:])
            if e > 0:
                _down(e - 1)
        _down(E - 1)
        yt_sb = sb.tile([128, KD, SUPER], MM_DT, tag="ytsb")
        for md in range(KD):
            nc.vector.tensor_copy(yt_sb[:, md, :], yt_ps[md][:])

        for s in range(S_BLK):
            o_ps = ps_big.tile([128, EMB], F32, tag="big")
            for kd in range(KD):
                nc.tensor.matmul(
                    o_ps[:],
                    yt_sb[:, kd, s * 128 : (s + 1) * 128],
                    wp_sb[:, kd, :],
                    start=(kd == 0),
                    stop=(kd == KD - 1),
                )
            o_sb = sb.tile([128, EMB], F32, tag="osb")
            if s % 2 == 0:
                nc.scalar.copy(o_sb[:], o_ps[:])
            else:
                nc.vector.tensor_copy(o_sb[:], o_ps[:])
            nc.sync.dma_start(out[tok0 + s * 128 : tok0 + (s + 1) * 128, :], o_sb[:])


_PROGRAM = None


def _build():
    global _PROGRAM
    if _PROGRAM is not None:
        return _PROGRAM
    nc = bacc.Bacc("TRN2", target_bir_lowering=False, debug=False, num_devices=NCORES)
    xt = nc.dram_tensor("xt", [D, NSH], F32, kind="ExternalInput").ap()
    wg = nc.dram_tensor("w_gate", [D, E], F32, kind="ExternalInput").ap()
    w1 = nc.dram_tensor("w1", [E, D, H], MM_DT, kind="ExternalInput").ap()
    w2 = nc.dram_tensor("w2", [E, H, D], MM_DT, kind="ExternalInput").ap()
    wp = nc.dram_tensor("w_proj", [D, EMB], MM_DT, kind="ExternalInput").ap()
    ident = nc.dram_tensor("ident", [128, 128], F32, kind="ExternalInput").ap()
    out = nc.dram_tensor("out", [NSH, EMB], F32, kind="ExternalOutput").ap()
    with tile.TileContext(nc) as tc, ExitStack() as ctx:
        _moe_body(ctx, tc, xt, wg, w1, w2, wp, ident, out)
    nc.compile()
    _PROGRAM = nc
    return nc


def _install_trace_shim():
    """Recreate the antenv.axon_hooks NTFF profile hook (missing in this image)."""
    import sys
    import types
    import contextlib
    import ctypes

    if "antenv.axon_hooks" in sys.modules:
        return
    so_path = "/opt/axon/libaxon_pjrt.so"
    lib = ctypes.CDLL(so_path)
    lib.axon_start_nrt_profile.argtypes = [ctypes.POINTER(ctypes.c_int64), ctypes.c_size_t]
    lib.axon_start_nrt_profile.restype = ctypes.c_int64
    lib.axon_stop_nrt_profile.argtypes = [ctypes.c_char_p]
    lib.axon_stop_nrt_profile.restype = ctypes.c_int64

    @contextlib.contextmanager
    def _hook(output_dir, device_ids):
        import jax

        jax.devices()
        if device_ids:
            ids = (ctypes.c_int64 * len(device_ids))(*device_ids)
            rc = lib.axon_start_nrt_profile(ids, len(device_ids))
        else:
            rc = lib.axon_start_nrt_profile(None, 0)
        if rc != 0:
            raise RuntimeError(f"axon_start_nrt_profile rc={rc}")
        try:
            yield
        finally:
            n = lib.axon_stop_nrt_profile(str(output_dir).encode())
            if n <= 0:
                print(f"profile: {n} ntff files written to {output_dir}")

    mod = types.ModuleType("antenv.axon_hooks")
    _state = {"hook": _hook}
    mod.get_axon_ntff_profile_hook = lambda: _state["hook"]
    mod.set_axon_ntff_profile_hook = lambda h: _state.__setitem__("hook", h)
    sys.modules["antenv.axon_hooks"] = mod

    import concourse.bass_utils as bu

    bu.upload_artifacts = lambda tmpdir: f"local:{tmpdir}"


def kernel(x, w_gate, w1, w2, w_proj, b_proj):
    nc = _build()
    import ml_dtypes

    bf16 = ml_dtypes.bfloat16
    ident = np.eye(128, dtype=np.float32)
    w1_b = np.ascontiguousarray(w1.astype(bf16))
    w2_b = np.ascontiguousarray(w2.astype(bf16))
    wp_b = np.ascontiguousarray(w_proj.astype(bf16))
    in_maps = [
        {
            "xt": np.ascontiguousarray(x[i * NSH : (i + 1) * NSH].T),
            "w_gate": np.ascontiguousarray(w_gate),
            "w1": w1_b,
            "w2": w2_b,
            "w_proj": wp_b,
            "ident": ident,
        }
        for i in range(NCORES)
    ]
    trace = bool(int(os.environ.get("MOE_TRACE", "0")))
    if trace:
        _install_trace_shim()
        import tempfile

        tmpdir = os.environ.get("MOE_TRACE_DIR") or tempfile.mkdtemp(prefix="moe_trace_")
        res = run_bass_kernel_spmd(
            nc, in_maps, list(range(NCORES)), trace=True, tmpdir=tmpdir,
            trace_cores=[0],
        )
        print(f"HW exec time: {res.exec_time_ns} ns")
        print(f"trace dir: {tmpdir}")
        kernel.last_result = res
    else:
        res = run_bass_kernel_spmd(nc, in_maps, list(range(NCORES)))
    full = np.concatenate([res.results[i]["out"] for i in range(NCORES)], axis=0)
    return full + b_proj[None, :]


# revision 18
# speedup vs baseline: 1.0735x; 1.0081x over previous
"""Trainium2 Bass kernel for MoE head adapter (top-2 of 4 experts + proj).

Computes, for full inputs x[65536,256], w_gate[256,4], w1[4,256,512],
w2[4,512,256], w_proj[256,512], b_proj[512]:

    logits = x @ w_gate; top-2 softmax gates
    h = gelu(x @ w1[e]); y = sum_e g_e * (h_e @ w2[e]); out = y @ w_proj + b_proj

Sharding: pure data-parallel over tokens across 8 NeuronCores (8192
tokens/core, weights replicated, no collectives).

Per-core structure (two phases, to keep the PE HAM clock warm and avoid
ACT activation-table reloads):
  Phase A: for all super-tiles: load x, PE-transpose to xT (f32r + f32
           copies), gating logits (exact f32), top-2 softmax gates,
           per-expert gate rows gT[e].
  Phase B: for all super-tiles: dense 4-expert up-proj / gelu*gate /
           down-proj accumulation (transposed orientation, 512-token
           moving operands, float32r) + output projection.
"""

import os
from contextlib import ExitStack

import numpy as np

import concourse.bass as bass
import concourse.tile as tile
from concourse import bacc, mybir
from concourse.bass_utils import run_bass_kernel_spmd

N, D, E, H, EMB = 65536, 256, 4, 512, 512
NCORES = 8
NSH = N // NCORES          # tokens per core
SUPER = 512                # tokens per super-tile
NSUP = NSH // SUPER
S_BLK = SUPER // 128       # 128-token sub-blocks per super-tile
KD = D // 128              # k-tiles over D
MH = H // 128              # m-tiles over H

F32 = mybir.dt.float32
AF = mybir.ActivationFunctionType
ALU = mybir.AluOpType
AX = mybir.AxisListType

# bf16 matmul operands: 1 cy/row on the PE + automatic fast-weight-load.
MM_DT = mybir.dt.bfloat16


def _moe_body(ctx: ExitStack, tc, xt, wg, w1, w2, wp, ident, out):
    nc = tc.nc

    const = ctx.enter_context(tc.tile_pool(name="const", bufs=1))
    keep = ctx.enter_context(tc.tile_pool(name="keep", bufs=1))
    sb = ctx.enter_context(tc.tile_pool(name="sb", bufs=2))
    ps_big = ctx.enter_context(tc.tile_pool(name="psbig", bufs=5, space="PSUM"))
    ps_yt = ctx.enter_context(tc.tile_pool(name="psyt", bufs=2, space="PSUM"))
    ps_sm = ctx.enter_context(tc.tile_pool(name="pssm", bufs=1, space="PSUM"))

    # --- replicated constants -------------------------------------------------
    w1_sb = const.tile([128, KD, E, H], MM_DT)
    w2_sb = const.tile([128, MH, E, D], MM_DT)
    wp_sb = const.tile([128, KD, EMB], MM_DT)
    wg_sb = const.tile([128, KD, E], F32)
    for k in range(KD):
        for e in range(E):
            nc.gpsimd.dma_start(
                w1_sb[:, k, e, :], w1[e, k * 128 : (k + 1) * 128, :]
            )
    for m in range(MH):
        for e in range(E):
            nc.gpsimd.dma_start(
                w2_sb[:, m, e, :], w2[e, m * 128 : (m + 1) * 128, :]
            )
    nc.gpsimd.dma_start(
        wp_sb[:], wp.rearrange("(k p) m -> p k m", p=128)
    )
    nc.gpsimd.dma_start(wg_sb[:], wg.rearrange("(k p) e -> p k e", p=128))
    id_sb = const.tile([128, 128], F32)
    nc.gpsimd.dma_start(id_sb[:], ident[:])
    ones_sb = const.tile([1, 128], MM_DT)
    nc.vector.memset(ones_sb[:], 1.0)

    # persistent across phases
    xt_all = keep.tile([128, NSUP, KD, SUPER], MM_DT)
    gte_dram = nc.dram_tensor("gte_scratch", [NSUP, E, SUPER], MM_DT).ap()

    # ======================= Phase A: gating =================================
    for T in range(NSUP):
        tok0 = T * SUPER

        xt32_sb = sb.tile([128, KD, SUPER], F32, tag="xt32", bufs=4)
        nc.sync.dma_start(
            xt32_sb[:],
            xt[:, tok0 : tok0 + SUPER].rearrange("(k p) t -> p k t", p=128),
        )
        nc.vector.tensor_copy(xt_all[:, T, :, :], xt32_sb[:])

        # gating logits [tok, s, e] in exact f32
        lg_ps = ps_sm.tile([128, S_BLK, E], F32, tag="sm")
        for s in range(S_BLK):
            for k in range(KD):
                nc.tensor.matmul(
                    lg_ps[:, s, :],
                    xt32_sb[:, k, s * 128 : (s + 1) * 128],
                    wg_sb[:, k, :],
                    start=(k == 0),
                    stop=(k == KD - 1),
                )

        # top-2 softmax gates
        def bc(t):
            return t[:].broadcast_to([128, S_BLK, E])

        lg = sb.tile([128, S_BLK, E], F32, tag="lg")
        nc.vector.tensor_copy(lg[:], lg_ps[:])
        m1 = sb.tile([128, S_BLK, 1], F32, tag="m1")
        nc.vector.reduce_max(m1[:], lg[:], axis=AX.X)
        t0 = sb.tile([128, S_BLK, E], F32, tag="t0")
        nc.vector.tensor_tensor(t0[:], lg[:], bc(m1), op=ALU.is_equal)
        t1 = sb.tile([128, S_BLK, E], F32, tag="t1")
        nc.vector.tensor_scalar_mul(t1[:], t0[:], -1e9)
        t2 = sb.tile([128, S_BLK, E], F32, tag="t2")
        nc.vector.tensor_tensor(t2[:], lg[:], t1[:], op=ALU.add)
        m2 = sb.tile([128, S_BLK, 1], F32, tag="m2")
        nc.vector.reduce_max(m2[:], t2[:], axis=AX.X)
        t3 = sb.tile([128, S_BLK, E], F32, tag="t3")
        nc.vector.tensor_tensor(t3[:], lg[:], bc(m2), op=ALU.is_ge)
        t4 = sb.tile([128, S_BLK, E], F32, tag="t4")
        nc.vector.tensor_tensor(t4[:], lg[:], bc(m1), op=ALU.subtract)
        t5 = sb.tile([128, S_BLK, E], F32, tag="t5")
        nc.scalar.activation(t5[:], t4[:], AF.Exp)
        t6 = sb.tile([128, S_BLK, E], F32, tag="t6")
        nc.vector.tensor_tensor(t6[:], t5[:], t3[:], op=ALU.mult)
        den = sb.tile([128, S_BLK, 1], F32, tag="den")
        nc.vector.reduce_sum(den[:], t6[:], axis=AX.X)
        rcp = sb.tile([128, S_BLK, 1], F32, tag="rcp")
        nc.vector.reciprocal(rcp[:], den[:])
        g_sb = sb.tile([128, S_BLK, E], F32, tag="g")
        nc.vector.tensor_tensor(g_sb[:], t6[:], bc(rcp), op=ALU.mult)

        # gT [e, tok] via one PE transpose per s-block, then partition-shift
        # DMAs to get each expert row based at partition 0.
        gt_ps = ps_sm.tile([E, SUPER], F32, tag="sm")
        for s in range(S_BLK):
            nc.tensor.transpose(
                gt_ps[:, s * 128 : (s + 1) * 128], g_sb[:, s, :], id_sb[:]
            )
        gt_sb = sb.tile([E, SUPER], MM_DT, tag="gtsb")
        nc.vector.tensor_copy(gt_sb[:], gt_ps[:])
        for e in range(E):
            nc.sync.dma_start(gte_dram[T, e, :], gt_sb[e : e + 1, :])

    # ======================= Phase B: experts + proj =========================
    for T in range(NSUP):
        tok0 = T * SUPER

        yt_ps = [
            ps_yt.tile([128, SUPER], F32, tag="yt", name=f"yt{T}_{md}")
            for md in range(KD)
        ]
        hgg_all = []

        def _down(e):
            hgg_e = hgg_all[e]
            for md in range(KD):
                for m in range(MH):
                    nc.tensor.matmul(
                        yt_ps[md][:],
                        w2_sb[:, m, e, md * 128 : (md + 1) * 128],
                        hgg_e[:, m, :],
                        start=(e == 0 and m == 0),
                        stop=(e == E - 1 and m == MH - 1),
                    )

        for e in range(E):
            gte_sb = sb.tile([1, SUPER], MM_DT, tag="gte")
            nc.sync.dma_start(gte_sb[:], gte_dram[T, e, :][None, :])
            G_ps = ps_big.tile([128, SUPER], F32, tag="big", name=f"G{T}_{e}")
            nc.tensor.matmul(
                G_ps[:], ones_sb[:], gte_sb[:], start=True, stop=True
            )
            G_sb = sb.tile([128, SUPER], MM_DT, tag="Gsb", name=f"Gsb{T}_{e}")
            nc.scalar.copy(G_sb[:], G_ps[:])
            hgg = sb.tile([128, MH, SUPER], MM_DT, tag="hgg", name=f"hgg{T}_{e}")
            hgg_all.append(hgg)
            for m in range(MH):
                h_ps = ps_big.tile([128, SUPER], F32, tag="big")
                for k in range(KD):
                    nc.tensor.matmul(
                        h_ps[:],
                        w1_sb[:, k, e, m * 128 : (m + 1) * 128],
                        xt_all[:, T, k, :],
                        start=(k == 0),
                        stop=(k == KD - 1),
                    )
                hg = sb.tile([128, SUPER], MM_DT, tag="hg")
                nc.scalar.activation(hg[:], h_ps[:], AF.Gelu)
                nc.vector.tensor_mul(hgg[:, m, :], hg[:], G_sb[:])
            if e > 0:
                _down(e - 1)
        _down(E - 1)
        yt_sb = sb.tile([128, KD, SUPER], MM_DT, tag="ytsb")
        for md in range(KD):
            nc.vector.tensor_copy(yt_sb[:, md, :], yt_ps[md][:])

        for s in range(S_BLK):
            o_ps = ps_big.tile([128, EMB], F32, tag="big")
            for kd in range(KD):
                nc.tensor.matmul(
                    o_ps[:],
                    yt_sb[:, kd, s * 128 : (s + 1) * 128],
                    wp_sb[:, kd, :],
                    start=(kd == 0),
                    stop=(kd == KD - 1),
                )
            o_sb = sb.tile([128, EMB], F32, tag="osb")
            if s % 2 == 0:
                nc.scalar.copy(o_sb[:], o_ps[:])
            else:
                nc.vector.tensor_copy(o_sb[:], o_ps[:])
            nc.sync.dma_start(out[tok0 + s * 128 : tok0 + (s + 1) * 128, :], o_sb[:])


_PROGRAM = None


def _build():
    global _PROGRAM
    if _PROGRAM is not None:
        return _PROGRAM
    nc = bacc.Bacc("TRN2", target_bir_lowering=False, debug=False, num_devices=NCORES)
    xt = nc.dram_tensor("xt", [D, NSH], F32, kind="ExternalInput").ap()
    wg = nc.dram_tensor("w_gate", [D, E], F32, kind="ExternalInput").ap()
    w1 = nc.dram_tensor("w1", [E, D, H], MM_DT, kind="ExternalInput").ap()
    w2 = nc.dram_tensor("w2", [E, H, D], MM_DT, kind="ExternalInput").ap()
    wp = nc.dram_tensor("w_proj", [D, EMB], MM_DT, kind="ExternalInput").ap()
    ident = nc.dram_tensor("ident", [128, 128], F32, kind="ExternalInput").ap()
    out = nc.dram_tensor("out", [NSH, EMB], F32, kind="ExternalOutput").ap()
    with tile.TileContext(nc) as tc, ExitStack() as ctx:
        _moe_body(ctx, tc, xt, wg, w1, w2, wp, ident, out)
    nc.compile()
    _PROGRAM = nc
    return nc


def _install_trace_shim():
    """Recreate the antenv.axon_hooks NTFF profile hook (missing in this image)."""
    import sys
    import types
    import contextlib
    import ctypes

    if "antenv.axon_hooks" in sys.modules:
        return
    so_path = "/opt/axon/libaxon_pjrt.so"
    lib = ctypes.CDLL(so_path)
    lib.axon_start_nrt_profile.argtypes = [ctypes.POINTER(ctypes.c_int64), ctypes.c_size_t]
    lib.axon_start_nrt_profile.restype = ctypes.c_int64
    lib.axon_stop_nrt_profile.argtypes = [ctypes.c_char_p]
    lib.axon_stop_nrt_profile.restype = ctypes.c_int64

    @contextlib.contextmanager
    def _hook(output_dir, device_ids):
        import jax

        jax.devices()
        if device_ids:
            ids = (ctypes.c_int64 * len(device_ids))(*device_ids)
            rc = lib.axon_start_nrt_profile(ids, len(device_ids))
        else:
            rc = lib.axon_start_nrt_profile(None, 0)
        if rc != 0:
            raise RuntimeError(f"axon_start_nrt_profile rc={rc}")
        try:
            yield
        finally:
            n = lib.axon_stop_nrt_profile(str(output_dir).encode())
            if n <= 0:
                print(f"profile: {n} ntff files written to {output_dir}")

    mod = types.ModuleType("antenv.axon_hooks")
    _state = {"hook": _hook}
    mod.get_axon_ntff_profile_hook = lambda: _state["hook"]
    mod.set_axon_ntff_profile_hook = lambda h: _state.__setitem__("hook", h)
    sys.modules["antenv.axon_hooks"] = mod

    import concourse.bass_utils as bu

    bu.upload_artifacts = lambda tmpdir: f"local:{tmpdir}"


def kernel(x, w_gate, w1, w2, w_proj, b_proj):
    nc = _build()
    import ml_dtypes

    bf16 = ml_dtypes.bfloat16
    ident = np.eye(128, dtype=np.float32)
    w1_b = np.ascontiguousarray(w1.astype(bf16))
    w2_b = np.ascontiguousarray(w2.astype(bf16))
    wp_b = np.ascontiguousarray(w_proj.astype(bf16))
    in_maps = [
        {
            "xt": np.ascontiguousarray(x[i * NSH : (i + 1) * NSH].T),
            "w_gate": np.ascontiguousarray(w_gate),
            "w1": w1_b,
            "w2": w2_b,
            "w_proj": wp_b,
            "ident": ident,
        }
        for i in range(NCORES)
    ]
    trace = bool(int(os.environ.get("MOE_TRACE", "0")))
    if trace:
        _install_trace_shim()
        import tempfile

        tmpdir = os.environ.get("MOE_TRACE_DIR") or tempfile.mkdtemp(prefix="moe_trace_")
        res = run_bass_kernel_spmd(
            nc, in_maps, list(range(NCORES)), trace=True, tmpdir=tmpdir,
            trace_cores=[0],
        )
        print(f"HW exec time: {res.exec_time_ns} ns")
        print(f"trace dir: {tmpdir}")
        kernel.last_result = res
    else:
        res = run_bass_kernel_spmd(nc, in_maps, list(range(NCORES)))
    full = np.concatenate([res.results[i]["out"] for i in range(NCORES)], axis=0)
    return full + b_proj[None, :]
